# revision 26
# baseline (speedup 1.0000x reference)
"""Trainium2 Bass kernel for DynamicConv2d (MoE-routed per-sample conv).

Data-parallel over batch: 32 samples -> 8 NeuronCores, 4 samples each.
Per core:
  - one-time: stream the 4-bank weight tensor in as bf16 via casting DMAs
    on the (otherwise idle) SWDGE queue, PE-transpose it into
    [cin, (3x3 tap, k, cout)] layout in SBUF, and fold sample 0's bank
    mixing into the same per-chunk pipeline.
  - per sample: load x, convert to a zero-padded bf16 image (channel sums
    for the router fall out of the same ScalarE pass via accum_out), run
    the router MLP + softmax on-chip, mix the banks into per-sample conv
    weights (ScalarE scaled-copy + DVE fused multiply-adds), then run the
    3x3 conv as 18 accumulating matmuls (2 cin chunks x 9 taps) per
    (cout chunk, 8-row tile) in bf16 with f32 PSUM accumulate.
"""

import numpy as np

import concourse.bass as bass
import concourse.tile as tile
from concourse import bacc, mybir
from concourse import bass_utils, masks

F32 = mybir.dt.float32
BF16 = mybir.dt.bfloat16
AF = mybir.ActivationFunctionType
ALU = mybir.AluOpType
AX = mybir.AxisListType

B, CIN, H, W = 32, 256, 56, 56
COUT, KB, KK = 256, 4, 3
HID = 64
N_CORES = 8
BL = B // N_CORES          # samples per core
CICH = CIN // 128          # cin chunks
OCCH = COUT // 128         # cout chunks
RT = 7                     # row tiles per image
RR = H // RT               # rows per tile (8)
NFREE = RR * W             # matmul free dim (448)
HP = H + 2                 # padded height (58)
WP = W + 2                 # padded width (58)


def _emit_router(nc, pools, state, b):
    """x load + channel-mean + router MLP + softmax for sample b.

    Returns the broadcast routing weights a_bc [128, KB]."""
    xstage_p, small_p, aux_psum = (
        pools["xstage"], pools["small"], pools["aux_psum"])
    x_ap = state["x_ap"]
    xpad = state["xpad"]
    fc1_wT, fc2_wT = state["fc1_wT"], state["fc2_wT"]
    fc1_b, fc2_b = state["fc1_b"], state["fc2_b"]

    par = b % 2
    v_sb = small_p.tile([128, CICH], F32, tag="v", name=f"v_{b}")
    xst = []
    for ci in range(CICH):
        t = xstage_p.tile([128, H * W], F32, tag="xs", name=f"xs_{b}_{ci}")
        nc.sync.dma_start(
            t[:], x_ap[b, ci * 128:(ci + 1) * 128].rearrange("c h w -> c (h w)"))
        xst.append(t)
    for ci in range(CICH):
        # f32 -> bf16 convert into the padded conv input (interior only;
        # borders were zeroed once at setup and are never written again);
        # accum_out gives the channel sums for the router in the same pass
        nc.scalar.activation(
            xpad[par][ci][:, 1:H + 1, 1:W + 1],
            xst[ci].rearrange("c (h w) -> c h w", w=W), AF.Copy,
            accum_out=v_sb[:, ci:ci + 1])

    # router MLP: h = relu(fc1_w @ (v/3136) + b1); logits = h @ fc2_wT + b2
    psum_h = aux_psum.tile([HID, 1], F32, tag="aux", bufs=1, name=f"ph_{b}")
    for ci in range(CICH):
        nc.tensor.matmul(psum_h[:], fc1_wT[:, ci, :], v_sb[:, ci:ci + 1],
                         start=(ci == 0), stop=(ci == CICH - 1))
    h_sb = small_p.tile([HID, 1], F32, tag="h", name=f"h_{b}")
    nc.scalar.activation(h_sb[:], psum_h[:], AF.Relu, bias=fc1_b[:])

    psum_l = aux_psum.tile([1, KB], F32, tag="aux", bufs=1, name=f"pl_{b}")
    nc.tensor.matmul(psum_l[:], h_sb[:], fc2_wT[:])
    logit = small_p.tile([1, KB], F32, tag="lg", name=f"lg_{b}")
    nc.vector.tensor_add(logit[:], psum_l[:], fc2_b[:])

    # softmax over the 4 banks
    nmax = small_p.tile([1, 1], F32, tag="nm", name=f"nm_{b}")
    nc.vector.reduce_max(nmax[:], logit[:], axis=AX.X, negate=True)
    e_sb = small_p.tile([1, KB], F32, tag="e", name=f"e_{b}")
    s_sb = small_p.tile([1, 1], F32, tag="s", name=f"s_{b}")
    nc.scalar.activation(e_sb[:], logit[:], AF.Exp, bias=nmax[:, 0:1],
                         accum_out=s_sb[:])
    r_sb = small_p.tile([1, 1], F32, tag="r", name=f"r_{b}")
    nc.vector.reciprocal(r_sb[:], s_sb[:])
    a_sb = small_p.tile([1, KB], F32, tag="a", name=f"a_{b}")
    nc.vector.tensor_scalar_mul(a_sb[:], e_sb[:], r_sb[:, 0:1])
    # broadcast across partitions with a tiny ones-matmul (PE is idle here
    # and this keeps the SWDGE queue free for the bank DMAs)
    psum_bc = aux_psum.tile([128, KB], F32, tag="aux", bufs=1, name=f"pb_{b}")
    nc.tensor.matmul(psum_bc[:], state["ones"][:], a_sb[:])
    a_bc = small_p.tile([128, KB], F32, tag="abc", name=f"abc_{b}")
    nc.vector.tensor_copy(a_bc[:], psum_bc[:])
    return a_bc


def _emit_mix_pass(nc, pools, state, b, ci, oc, k, a_bc, wa_map):
    """One bank-mixing pass: wa += a[k] * bankT[:, :, k, oc half]."""
    wacc_p, wdyn_p = pools["wacc"], pools["wdyn"]
    bankT = state["bankT"]
    osl = slice(oc * 128, (oc + 1) * 128)
    if k == 0:
        wa = wacc_p.tile([128, KK * KK, 128], F32, tag="wa",
                         name=f"wa_{b}_{ci}_{oc}")
        nc.scalar.activation(wa[:], bankT[ci][:, :, 0, osl], AF.Copy,
                             scale=a_bc[:, 0:1])
        wa_map[(ci, oc)] = wa
        return None
    eng = nc.vector
    wa = wa_map[(ci, oc)]
    if k < KB - 1:
        eng.scalar_tensor_tensor(
            wa[:], bankT[ci][:, :, k, osl], a_bc[:, k:k + 1], wa[:],
            op0=ALU.mult, op1=ALU.add)
        return None
    wd = wdyn_p.tile([128, KK * KK, 128], BF16, tag="wd",
                     name=f"wd_{b}_{ci}_{oc}")
    eng.scalar_tensor_tensor(
        wd[:], bankT[ci][:, :, k, osl], a_bc[:, k:k + 1], wa[:],
        op0=ALU.mult, op1=ALU.add)
    return wd


def _emit_mixing(nc, pools, state, b, a_bc):
    """All mixing passes for one sample (samples 1+; sample 0 interleaves
    its passes with the bank-chunk pipeline instead)."""
    wdyn, wa_map = {}, {}
    for ci in range(CICH):
        for oc in range(OCCH):
            for k in range(KB):
                wd = _emit_mix_pass(nc, pools, state, b, ci, oc, k, a_bc,
                                    wa_map)
                if wd is not None:
                    wdyn[(ci, oc)] = wd
    return wdyn


def _emit_sample_conv(nc, pools, state, b, wdyn, oc_list=(0, 1)):
    """3x3 conv with the per-sample mixed weights; writes y[b]."""
    py_p, ysb_p = pools["py_psum"], pools["ysb"]
    y_ap = state["y_ap"]
    xpad = state["xpad"]
    par = b % 2
    for oc in oc_list:
        py = [py_p.tile([128, NFREE], F32, tag="py", name=f"py_{b}_{oc}_{rt}")
              for rt in range(RT)]
        for ci in range(CICH):
            for dij in range(KK * KK):
                di, dj = dij // KK, dij % KK
                lhsT = wdyn[(ci, oc)][:, dij, :]
                first = (ci == 0 and dij == 0)
                last = (ci == CICH - 1 and dij == KK * KK - 1)
                for rt in range(RT):
                    rhs = xpad[par][ci][:, rt * RR + di: rt * RR + di + RR,
                                        dj: dj + W]
                    nc.tensor.matmul(py[rt][:], lhsT, rhs,
                                     start=first, stop=last)
        for rt in range(RT):
            ysb = ysb_p.tile([128, NFREE], F32, tag="ysb",
                             name=f"ysb_{b}_{oc}_{rt}")
            if b == BL - 1 and rt % 2 == 1:
                nc.scalar.activation(ysb[:], py[rt][:], AF.Copy)
            else:
                nc.vector.tensor_copy(ysb[:], py[rt][:])
            nc.sync.dma_start(
                y_ap[b, oc * 128:(oc + 1) * 128, rt * RR:(rt + 1) * RR, :],
                ysb.rearrange("c (h w) -> c h w", w=W))


def build_kernel(nc, tc, x_ap, wb_ap, fc1w_ap, fc1b_ap, fc2w_ap, fc2b_ap, y_ap):
    const_p = tc.alloc_tile_pool(name="const", bufs=1)
    pools = {
        "xstage": tc.alloc_tile_pool(name="xstage", bufs=3),
        "wacc": tc.alloc_tile_pool(name="wacc", bufs=4),
        "wdyn": tc.alloc_tile_pool(name="wdyn", bufs=8),
        "small": tc.alloc_tile_pool(name="small", bufs=2),
        "ysb": tc.alloc_tile_pool(name="ysb", bufs=8),
    }
    pools["aux_psum"] = tc.alloc_tile_pool(name="aux_psum", bufs=1,
                                           space="PSUM")

    # ---- constants -------------------------------------------------------
    ident = const_p.tile([128, 128], BF16, name="ident")
    masks.make_identity(nc, ident[:])
    ident32 = const_p.tile([128, 128], F32, name="ident32")
    masks.make_identity(nc, ident32[:])
    ones = const_p.tile([1, 128], F32, name="ones")
    nc.vector.memset(ones[:], 1.0)

    fc1_nat = const_p.tile([HID, CIN], F32, name="fc1_nat")
    nc.sync.dma_start(fc1_nat[:], fc1w_ap)
    fc2_nat = const_p.tile([KB, HID], F32, name="fc2_nat")
    nc.sync.dma_start(fc2_nat[:], fc2w_ap)
    fc1_b = const_p.tile([HID, 1], F32, name="fc1_b")
    nc.sync.dma_start(fc1_b[:], fc1b_ap.unsqueeze(1))
    fc2_b = const_p.tile([1, KB], F32, name="fc2_b")
    nc.sync.dma_start(fc2_b[:], fc2b_ap.unsqueeze(0))

    # persistent padded conv inputs (2 parities x 2 cin chunks);
    # zero once -> borders stay zero, interiors rewritten per sample
    xpad = [[const_p.tile([128, HP, WP], BF16, name=f"xpad_{p}_{ci}")
             for ci in range(CICH)] for p in range(2)]
    for p in range(2):
        for ci in range(CICH):
            nc.vector.memset(xpad[p][ci][:], 0.0)

    state = {"x_ap": x_ap, "y_ap": y_ap, "xpad": xpad, "ones": ones,
             "fc1_b": fc1_b, "fc2_b": fc2_b}

    # ---- bank stream-in: casting DMAs on the SWDGE queue -----------------
    # bankT[ci] layout: [cin(128), 3x3 tap, bank k, cout] in bf16
    setup_p = tc.alloc_tile_pool(name="setup", bufs=3)
    tp_psum = tc.alloc_tile_pool(name="tp_psum", bufs=3, space="PSUM")
    bankT = [const_p.tile([128, KK * KK, KB, COUT], BF16, name=f"bankT_{ci}")
             for ci in range(CICH)]
    state["bankT"] = bankT
    wb_flat = wb_ap.rearrange("k o i h w -> (k o) (i h w)")
    nat16s = []
    for koc in range(KB * OCCH):
        nat16 = setup_p.tile([128, CIN * KK * KK], BF16, tag="nat16",
                             name=f"nat16_{koc}")
        if koc % 2 == 0 and koc < 6:
            # even chunks: SWDGE casting DMA (f32 DRAM -> bf16 SBUF)
            nc.gpsimd.dma_start(nat16[:],
                                wb_flat[koc * 128:(koc + 1) * 128, :])
        else:
            # odd chunks: HWDGE f32 load + DVE cast, in parallel with SWDGE
            nat = setup_p.tile([128, CIN * KK * KK], F32, tag="nat", bufs=2,
                              name=f"nat_{koc}")
            nc.sync.dma_start(nat[:], wb_flat[koc * 128:(koc + 1) * 128, :])
            nc.vector.tensor_copy(nat16[:], nat[:])
        nat16s.append(nat16)

    # fc1_wT[:, ci, :] = fc1_w[:, ci-chunk].T / (H*W)   (folds the mean)
    fc1_wT = const_p.tile([128, CICH, HID], F32, name="fc1_wT")
    for ci in range(CICH):
        pt = pools["aux_psum"].tile([128, HID], F32, tag="aux", bufs=1,
                                    name=f"fc1t_{ci}")
        nc.tensor.transpose(pt[:], fc1_nat[:, ci * 128:(ci + 1) * 128],
                            ident32[:HID, :HID])
        nc.scalar.activation(fc1_wT[:, ci, :], pt[:], AF.Copy,
                             scale=1.0 / float(H * W))
    fc2_wT = const_p.tile([HID, KB], F32, name="fc2_wT")
    pt2 = pools["aux_psum"].tile([HID, KB], F32, tag="aux", bufs=1,
                                 name="fc2t")
    nc.tensor.transpose(pt2[:], fc2_nat[:], ident32[:KB, :KB])
    nc.vector.tensor_copy(fc2_wT[:], pt2[:])
    state["fc1_wT"], state["fc2_wT"] = fc1_wT, fc2_wT

    # ---- router for sample 0 ---------------------------------------------
    a_bc0 = _emit_router(nc, pools, state, 0)

    # ---- per-chunk transpose + copy + sample-0 mixing pass ---------------
    # chunk (k, oc2) holds bank k's weights for cout half oc2; as soon as a
    # chunk is transposed into bankT, sample 0's mixing pass k for that
    # cout half runs, so mix(0) finishes right behind the last chunk
    wdyn0, wa0 = {}, {}
    for koc in range(KB * OCCH):
        k, oc2 = koc // OCCH, koc % OCCH
        natv = nat16s[koc].rearrange("p (i hw) -> p i hw", hw=KK * KK)
        pts = []
        for ci in range(CICH):
            pt = tp_psum.tile([128, KK * KK, 128], BF16, tag="tp",
                              name=f"tp_{koc}_{ci}")
            for hw in range(KK * KK):
                nc.tensor.transpose(
                    pt[:, hw, :], natv[:, ci * 128:(ci + 1) * 128, hw],
                    ident[:])
            pts.append(pt)
        for ci in range(CICH):
            # one consolidated copy per (chunk, cin half)
            if ci == 0:
                nc.vector.tensor_copy(
                    bankT[ci][:, :, k, oc2 * 128:(oc2 + 1) * 128], pts[ci][:])
            else:
                nc.scalar.activation(
                    bankT[ci][:, :, k, oc2 * 128:(oc2 + 1) * 128], pts[ci][:],
                    AF.Copy)
        for ci in range(CICH):
            wd = _emit_mix_pass(nc, pools, state, 0, ci, oc2, k, a_bc0, wa0)
            if wd is not None:
                wdyn0[(ci, oc2)] = wd
    # the PE is forced idle here while DVE finishes mix(0); keep the HAM
    # activity monitor warm with throwaway matmuls so conv(0) starts at
    # full clock instead of 1.2 GHz
    for wi in range(16):
        wt = pools["aux_psum"].tile([128, 128], F32, tag="aux", bufs=1,
                                    name=f"warm_{wi}")
        nc.tensor.matmul(wt[:], ident32[:], ident32[:])
    tp_psum.release()
    setup_p.release()

    pools["py_psum"] = tc.alloc_tile_pool(name="py_psum", bufs=RT,
                                          space="PSUM")

    # ---- software-pipelined per-sample loop ------------------------------
    # router+mixing for sample b+1 is emitted before conv(b) so the PE never
    # waits on the DVE mixing.
    wdyn_q = {0: wdyn0}
    for b in range(BL):
        if b + 1 < BL:
            a_bc = _emit_router(nc, pools, state, b + 1)
            wdyn_q[b + 1] = _emit_mixing(nc, pools, state, b + 1, a_bc)
        _emit_sample_conv(nc, pools, state, b, wdyn_q.pop(b))

    for name in ("py_psum", "aux_psum", "ysb", "small", "wdyn", "wacc",
                 "xstage"):
        pools[name].release()
    const_p.release()


_NC_CACHE = {}


def _build():
    nc = bacc.Bacc("TRN2", target_bir_lowering=False, debug=False,
                   enable_asserts=False)
    x_d = nc.dram_tensor("x", [BL, CIN, H, W], F32, kind="ExternalInput")
    wb_d = nc.dram_tensor("weight_bank", [KB, COUT, CIN, KK, KK], F32,
                          kind="ExternalInput")
    fc1w_d = nc.dram_tensor("fc1_w", [HID, CIN], F32, kind="ExternalInput")
    fc1b_d = nc.dram_tensor("fc1_b", [HID], F32, kind="ExternalInput")
    fc2w_d = nc.dram_tensor("fc2_w", [KB, HID], F32, kind="ExternalInput")
    fc2b_d = nc.dram_tensor("fc2_b", [KB], F32, kind="ExternalInput")
    y_d = nc.dram_tensor("y", [BL, COUT, H, W], F32, kind="ExternalOutput")
    with tile.TileContext(nc) as tc:
        build_kernel(nc, tc, x_d.ap(), wb_d.ap(), fc1w_d.ap(), fc1b_d.ap(),
                     fc2w_d.ap(), fc2b_d.ap(), y_d.ap())
    nc.compile()
    return nc


def get_nc():
    if "nc" not in _NC_CACHE:
        _NC_CACHE["nc"] = _build()
    return _NC_CACHE["nc"]


def make_in_maps(x, weight_bank, fc1_w, fc1_b, fc2_w, fc2_b):
    x = np.ascontiguousarray(np.asarray(x, dtype=np.float32))
    rep = {
        "weight_bank": np.ascontiguousarray(np.asarray(weight_bank, np.float32)),
        "fc1_w": np.ascontiguousarray(np.asarray(fc1_w, np.float32)),
        "fc1_b": np.ascontiguousarray(np.asarray(fc1_b, np.float32)),
        "fc2_w": np.ascontiguousarray(np.asarray(fc2_w, np.float32)),
        "fc2_b": np.ascontiguousarray(np.asarray(fc2_b, np.float32)),
    }
    return [dict(rep, x=np.ascontiguousarray(x[c * BL:(c + 1) * BL]))
            for c in range(N_CORES)]


def kernel(x, weight_bank, fc1_w, fc1_b, fc2_w, fc2_b):
    nc = get_nc()
    in_maps = make_in_maps(x, weight_bank, fc1_w, fc1_b, fc2_w, fc2_b)
    res = bass_utils.run_bass_kernel_spmd(nc, in_maps,
                                          core_ids=list(range(N_CORES)))
    return np.concatenate([r["y"] for r in res.results], axis=0)
